# revision 24
# baseline (speedup 1.0000x reference)
"""BasisVQ kernel for 8 Trainium2 NeuronCores.

Math (forward values only):
  z      = x @ W.T + b            [BK, 2700]   (b is zero in practice)
  s_c    = ||z||^2 - 2 z.b_c + ||b_c||^2      (squared distance)
  Define s_neg_c = 2 z.b_c - ||b_c||^2 = -(s_c - ||z||^2):
    argmin_c s_c      == argmax_c s_neg_c
    softmax(-s)       == softmax(s_neg)        (shift invariance)
    min_c s_c         == ||z||^2 - max_c s_neg_c
  q_st forward value == basis[argmin]
  commit mean        == mean_t(||z_t||^2 + min-part) ; vq_loss = 0.25 * commit
  entropy            from avg softmax probs.

Device strategy (data-parallel over the 32768 tokens, 4096/core):
  matmul1: z2 = (2W x) accumulated fp32 in PSUM, fp32r inputs (11-bit mantissa,
           full PE rate). Stored as z2 = 2z in SBUF, [basis_dim, tokens] layout.
  dist:    PSUM[t, c] = sum_e z2[e,t]*basisT[e,c] - ||b_c||^2 via an augmented
           contraction row (ones row in z2, -||b||^2 row in basisT).
  epilogue: DVE max8+index -> argmin + top8; ACT exp(bias=-max, accum_out=sum);
           normalize, accumulate avg-prob partials; indirect-DMA gather of
           basis rows -> q_st. Scalar partial sums are combined on host.
"""

import os
import numpy as np

TOKENS = 32768
DM = 256
BD = 2700
CODES = 1024
N_CORES = 8
TPC = TOKENS // N_CORES      # 4096 tokens per core
T = 256                      # tokens per block
CH = 512                     # codes per psum chunk
NE = 22                      # e-tiles: 21*128 + 12
E_LAST = 12                  # rows in last e tile (aug rows add AUG)
AUG = 2                      # hi/lo split of -||b||^2 (fp32r can't hold ~900 exactly)

_PROGRAMS = {}
LAST_RESULTS = None


def _r11(a):
    """fp32 -> fp32r rounding (round-to-nearest, 11-bit mantissa), as the
    Trainium cast-DMA does it (verified on HW)."""
    u = np.ascontiguousarray(a, dtype=np.float32).view(np.uint32).astype(np.uint64)
    u = ((u + 0x800) & 0xFFFFF000).astype(np.uint32)
    return u.view(np.float32)


def _build_program(tpc, with_bias, ps1_bufs=4, pss_bufs=2, zs_bufs=2, reps=1):
    from contextlib import ExitStack
    import concourse.bass as bass
    import concourse.mybir as mybir
    import concourse.tile as tile
    from concourse import bacc

    f32 = mybir.dt.float32
    f32r = mybir.dt.float32r
    u32 = mybir.dt.uint32
    ADD = mybir.AluOpType.add
    EXP = mybir.ActivationFunctionType.Exp
    LN = mybir.ActivationFunctionType.Ln
    SUB = mybir.AluOpType.subtract
    SQ = mybir.ActivationFunctionType.Square
    X = mybir.AxisListType.X

    nblk = tpc // T
    ntt = T // 128
    n_tt_total = tpc // 128

    nc = bacc.Bacc("TRN2", target_bir_lowering=False, num_swdge_queues=4)

    xT_d = nc.dram_tensor("xT", [DM, tpc], f32r, kind="ExternalInput")
    w2_d = nc.dram_tensor("w2T", [DM, BD], f32r, kind="ExternalInput")
    bTa_d = nc.dram_tensor("bTa", [BD + AUG, CODES], f32r, kind="ExternalInput")
    basis_d = nc.dram_tensor("basis", [CODES, BD], f32, kind="ExternalInput")
    if with_bias:
        b2_d = nc.dram_tensor("b2T", [1, BD], f32, kind="ExternalInput")
    qst_d = nc.dram_tensor("qst", [tpc, BD], f32, kind="ExternalOutput")
    idx_d = nc.dram_tensor("idx", [tpc], u32, kind="ExternalOutput")
    probs_d = nc.dram_tensor("probs", [128, CODES], f32, kind="ExternalOutput")
    stats_d = nc.dram_tensor("stats", [128, 2], f32, kind="ExternalOutput")

    with tile.TileContext(nc) as tc, ExitStack() as ctx:
        const = ctx.enter_context(tc.tile_pool(name="const", bufs=1))
        zsp = ctx.enter_context(tc.tile_pool(name="zsp", bufs=zs_bufs))
        xp = ctx.enter_context(tc.tile_pool(name="xp", bufs=4))
        gp = ctx.enter_context(tc.tile_pool(name="gp", bufs=2))
        pnp = ctx.enter_context(tc.tile_pool(name="pnp", bufs=2))
        smp = ctx.enter_context(tc.tile_pool(name="smp", bufs=2))
        scp = ctx.enter_context(tc.tile_pool(name="scp", bufs=1))
        ps1p = ctx.enter_context(tc.tile_pool(name="ps1p", bufs=ps1_bufs, space="PSUM"))
        pssp = ctx.enter_context(tc.tile_pool(name="pssp", bufs=pss_bufs, space="PSUM"))

        # --- resident constants ---
        w2 = [const.tile([128, BD], f32r, tag=f"w2_{k}", name=f"w2_{k}")
              for k in range(2)]
        bt = []
        for e in range(NE):
            rows = 128 if e < NE - 1 else E_LAST + AUG
            t = const.tile([rows, CODES], f32r, tag=f"bt_{e}", name=f"bt_{e}")
            nc.sync.dma_start(t[:], bTa_d[e * 128:e * 128 + rows, :])
            bt.append(t)
        if with_bias:
            b2 = const.tile([1, BD], f32r, tag="b2", name="b2")
            nc.gpsimd.dma_start(b2[:], b2_d[:])
            onesr_f = const.tile([1, T], f32, tag="onesr_f", name="onesr_f")
            nc.vector.memset(onesr_f[:], 1.0)
            onesr = const.tile([1, T], f32r, tag="onesr", name="onesr")
            nc.gpsimd.dma_start(onesr[:], onesr_f[:])

        # --- persistent state ---
        acc_p = const.tile([128, CODES], f32, tag="acc_p", name="acc_p")
        nc.vector.memset(acc_p[:], 0.0)
        acc_zq = const.tile([128, 1], f32, tag="acc_zq", name="acc_zq")
        nc.vector.memset(acc_zq[:], 0.0)
        acc_sm = const.tile([128, 1], f32, tag="acc_sm", name="acc_sm")
        nc.vector.memset(acc_sm[:], 0.0)
        idx_st = const.tile([128, n_tt_total], u32, tag="idx_st", name="idx_st")

        aug1 = const.tile([AUG, T], f32r, tag="aug1", name="aug1")
        nc.vector._memset_packed(aug1[:].bitcast(u32), 0x3F800000)

        def alloc_zs(b):
            zs = []
            for e in range(NE):
                rows = 128 if e < NE - 1 else E_LAST + AUG
                zs.append(zsp.tile([rows, T], f32r, tag=f"zs_{e}", name=f"zs_{e}_{b}"))
            # augmentation rows: ones, multiply the -||b||^2 rows of bTa.
            # (engine APs can't start at partition 12, DMA can)
            nc.scalar.dma_start(zs[NE - 1][E_LAST:E_LAST + AUG, :], aug1[:])
            return zs

        zs = alloc_zs(0) if zs_bufs == 1 else None

        def load_xt(b):
            lst = []
            for k in range(2):
                t = xp.tile([128, T], f32r, tag=f"x_{k}", name=f"x_{k}_{b}_r")
                nc.scalar.dma_start(t[:], xT_d[k * 128:(k + 1) * 128, b * T:(b + 1) * T])
                lst.append(t)
            return lst

        PREFETCH = 3
        xt_q = [load_xt(0)]
        for k in range(2):
            nc.scalar.dma_start(w2[k][:], w2_d[k * 128:(k + 1) * 128, :])
        xt_q += [load_xt(b) for b in range(1, min(PREFETCH, nblk))]

        for rep in range(reps):
          for b in range(nblk):
            if zs_bufs > 1:
                zs = alloc_zs(b + rep * nblk)
            xt = xt_q[b] if rep == 0 else load_xt(b)
            if rep == 0 and b + PREFETCH < nblk:
                xt_q.append(load_xt(b + PREFETCH))
            zq_stage = smp.tile([128, NE], f32, tag="zq_stage", name=f"zq_stage_{rep}_{b}")
            nc.vector.memset(zq_stage[:], 0.0)

            # matmul1: z2 = (2 W^T)^T x per e-tile, fp32r
            for e in range(NE):
                rows = 128 if e < NE - 1 else E_LAST
                es = slice(e * 128, e * 128 + rows)
                ps1 = ps1p.tile([rows, T], f32, tag="ps1", name=f"ps1_{rep}_{b}_{e}")
                nc.tensor.matmul(ps1[:], w2[0][:, es],
                                 xt[0][:], start=True, stop=False)
                nc.tensor.matmul(ps1[:], w2[1][:, es],
                                 xt[1][:], start=False,
                                 stop=not with_bias)
                if with_bias:
                    nc.tensor.matmul(ps1[:], b2[0:1, es],
                                     onesr[:], start=False, stop=True)
                if e % 2 == 0:
                    nc.vector.tensor_copy(zs[e][0:rows, :], ps1[:])
                else:
                    nc.scalar.copy(zs[e][0:rows, :], ps1[:])

            # dist + epilogue per 128-token tile
            for tt in range(ntt):
                gtt = b * ntt + tt
                tts = slice(tt * 128, (tt + 1) * 128)
                ps_s = pssp.tile([128, 2 * CH], f32, tag="ps_s", name=f"ps_s_{rep}_{gtt}")
                for c in range(2):
                    cs = slice(c * CH, (c + 1) * CH)
                    for e in range(NE):
                        nc.tensor.matmul(ps_s[:, cs], zs[e][:, tts],
                                         bt[e][:, cs],
                                         start=(e == 0), stop=(e == NE - 1))
                val8 = smp.tile([128, 8], f32, tag="val8", name=f"val8_{rep}_{gtt}")
                idx8 = smp.tile([128, 8], u32, tag="idx8", name=f"idx8_{rep}_{gtt}")
                nc.vector.max(val8[:], ps_s[:])
                nc.vector.max_index(idx8[:], val8[:], ps_s[:])
                mn = smp.tile([128, 1], f32, tag="mn", name=f"mn_{rep}_{gtt}")
                nc.vector.tensor_scalar_mul(mn[:], val8[:, 0:1], -1.0)
                pn = pnp.tile([128, 2 * CH], f32, tag="pn", name=f"pn_{rep}_{gtt}")
                sume = smp.tile([128, 1], f32, tag="sume", name=f"sume_{rep}_{gtt}")
                nc.scalar.activation(pn[:], ps_s[:], EXP, bias=mn[:], scale=1.0,
                                     accum_out=sume[:])
                rin = smp.tile([128, 1], f32, tag="rin", name=f"rin_{rep}_{gtt}")
                nc.vector.reciprocal(rin[:], sume[:])
                nc.scalar.mul(pn[:], pn[:], rin[:])
                nc.gpsimd.tensor_tensor(acc_p[:], acc_p[:], pn[:], op=ADD)
                nc.vector.tensor_tensor(acc_sm[:], acc_sm[:], val8[:, 0:1], op=ADD)
                nc.vector.tensor_copy(idx_st[:, gtt:gtt + 1], idx8[:, 0:1])
                g = gp.tile([128, BD], f32, tag="g", name=f"g_{rep}_{gtt}")
                nc.gpsimd.indirect_dma_start(
                    out=g[:], out_offset=None, in_=basis_d[:],
                    in_offset=bass.IndirectOffsetOnAxis(ap=idx8[:, 0:1], axis=0))
                nc.sync.dma_start(qst_d[gtt * 128:(gtt + 1) * 128, :], g[:])

            # sum of z2^2 off the critical path: read zs from SBUF at block end
            for e in range(NE):
                rows = 128 if e < NE - 1 else E_LAST
                sq = scp.tile([128, T], f32, tag="sq", name=f"sq_{rep}_{b}_{e}")
                nc.scalar.activation(sq[0:rows, :], zs[e][0:rows, :].bitcast(f32), SQ,
                                     accum_out=zq_stage[0:rows, e:e + 1])
            zqs = smp.tile([128, 1], f32, tag="zqs", name=f"zqs_{rep}_{b}")
            nc.vector.reduce_sum(zqs[:], zq_stage[:], axis=X)
            nc.vector.tensor_tensor(acc_zq[:], acc_zq[:], zqs[:], op=ADD)

        nc.sync.dma_start(probs_d[:], acc_p[:])
        nc.sync.dma_start(stats_d[:, 0:1], acc_zq[:])
        nc.sync.dma_start(stats_d[:, 1:2], acc_sm[:])
        nc.sync.dma_start(idx_d[:].rearrange("(i p) -> p i", p=128), idx_st[:])

    nc.compile()
    return nc


def _get_program(tpc, with_bias, **kw):
    key = (tpc, with_bias, tuple(sorted(kw.items())))
    if key not in _PROGRAMS:
        _PROGRAMS[key] = _build_program(tpc, with_bias, **kw)
    return _PROGRAMS[key]


def kernel(slot_features, W_proj, b_proj, basis):
    global LAST_RESULTS
    from concourse.bass_utils import run_bass_kernel_spmd

    B, K, _ = slot_features.shape
    x = np.ascontiguousarray(slot_features.reshape(B * K, DM), dtype=np.float32)
    xT = _r11(np.ascontiguousarray(x.T))                               # [256, 32768]
    w2T = _r11(np.ascontiguousarray(2.0 * W_proj.T, dtype=np.float32))
    basis = np.ascontiguousarray(basis, dtype=np.float32)
    bn = (basis * basis).sum(axis=1, dtype=np.float32)                 # [1024]
    # -||b||^2 ~ -900 exceeds fp32r (11-bit) resolution; split hi+lo so the
    # cast-DMA rounding is lossless
    nb = (-bn).astype(np.float64)
    hi = _r11(nb.astype(np.float32))
    lo = (nb - hi.astype(np.float64)).astype(np.float32)
    bTa = np.empty((BD + AUG, CODES), dtype=np.float32)
    bTa[:BD] = basis.T
    bTa[BD] = hi
    bTa[BD + 1] = lo
    bTa = _r11(np.ascontiguousarray(bTa))

    with_bias = bool(np.any(b_proj))
    nc = _get_program(TPC, with_bias)

    in_maps = []
    for c in range(N_CORES):
        m = {
            "xT": np.ascontiguousarray(xT[:, c * TPC:(c + 1) * TPC]),
            "w2T": w2T,
            "bTa": bTa,
            "basis": basis,
        }
        if with_bias:
            m["b2T"] = np.ascontiguousarray(
                2.0 * np.asarray(b_proj, dtype=np.float32).reshape(1, BD))
        in_maps.append(m)

    res = run_bass_kernel_spmd(nc, in_maps, core_ids=list(range(N_CORES)))
    LAST_RESULTS = res
    outs = res.results

    q_st = np.concatenate([r["qst"] for r in outs], axis=0).reshape(B, K, BD)
    indices = np.concatenate([r["idx"].view(np.int32) for r in outs]).reshape(B, K)

    probs_total = np.zeros(CODES, dtype=np.float64)
    zsq_total = 0.0
    smax_total = 0.0
    for r in outs:
        probs_total += r["probs"].astype(np.float64).sum(axis=0)
        zsq_total += float(r["stats"][:, 0].astype(np.float64).sum())
        smax_total += float(r["stats"][:, 1].astype(np.float64).sum())

    n_tok = B * K
    commit_sum = 0.25 * zsq_total - smax_total   # sum_t ||z_t||^2 + min_c s
    vq_loss = np.float32(0.25 * commit_sum / (n_tok * BD))
    avg_probs = probs_total / n_tok
    entropy = np.float32(-(avg_probs * np.log(avg_probs + 1e-8)).sum())

    return q_st, indices, vq_loss, entropy
